# revision 12
# baseline (speedup 1.0000x reference)
"""Distributed DMPNN (2-layer GRU message passing) for 8 TRN2 NeuronCores.

v6: dense-count-matrix formulation. Per-pass structure (steady state
measured via AG-to-AG period): two PE-bound M^T streams (~170 us each at
the warm 213 ns/MM N=512 roofline, mt DMA ~340 GB/s underneath) plus a
GRU/AllGather bridge that this version minimizes.

Math (linearity of segment_sum):
    msg  = concat(x[src], ea) @ Wm^T = y[src] + ea @ We^T,  y = x @ Wx^T
    agg  = seg_sum(msg, dst) = M @ y + (seg_sum(ea, dst)) @ We^T
with M[dst, src] the edge-multiplicity count matrix (fp8-exact).

Host precomputes (input-only): y1 = x @ Wx1^T (node-partitioned chunk
table), A^T = seg_sum(ea, dst)^T, and M^T grouped as [NG, 128, G, 2560]
(G*2560 contiguous bytes per partition per DMA). The final output is
produced feature-major on device and transposed on host.

Device per pass (dst nodes block-sharded, 2560/core):
  S1: aggT1 = We1-term + 20-group M^T stream (y1 ring DMA feeds lhsT)
  G1: feature-major GRU per dst tile (6 N=128 MMs; rT|zT|inT|hnT in one
      PSUM bank) writing x1T directly -- no PE transposes. After tiles
      0-3 the first AllGather of y2n = x1 @ Wx2^T fires; splits A/B/C =
      tiles 0-3 / 4-11 / 12-19 pipeline the collectives behind the GRU
      and behind S2-A streaming (host permutes mtg/y1n to match; fp8
      payload was tried: rel err 9e-2, fails the 2e-2 gate).
  S2: 20-group M^T stream, lhsT straight from the gathered ytab tables
  G2: feature-major GRU -> outT (f32) -> one DMA; host transposes.

PSUM budget (8 banks): 5 agg accumulators + 2 GRU/y2n banks + 1 spare.
"""

import numpy as np

import concourse.bass as bass
import concourse.mybir as mybir
import concourse.bacc as bacc
import concourse.tile as tile

F32 = mybir.dt.float32
BF16 = mybir.dt.float16  # 16-bit compute dtype (fp16: more mantissa than bf16)
F8 = mybir.dt.float8e4
NPBF16 = np.dtype(mybir.dt.np(BF16))
NPF8 = np.dtype(mybir.dt.np(F8))

N_CORES = 8
P = 128
N_NODES = 20000
NPC = 2500                 # valid nodes per core
NPC_PAD = 2560             # padded nodes per core (20 tiles)
NT = NPC_PAD // P          # dst tiles per core = 20
N_PAD = NPC_PAD * N_CORES  # 20480 device node ids
NCH = N_PAD // P           # src chunks = 160
NB = NPC_PAD // 512        # psum dst blocks = 5
ED = 64                    # edge feature dim
G = 8                      # src chunks per mt DMA group
NG = NCH // G              # mt DMA groups = 20
TSPLIT = (0, 4, 12, 20)    # dst-tile boundaries of the 3 AllGather waves


def chunk_perm():
    """Stream-2 processing order: wave A = every core's tiles 0..3, then
    wave B = tiles 4..11, then wave C = tiles 12..19."""
    out = []
    for s in range(3):
        t0, t1 = TSPLIT[s], TSPLIT[s + 1]
        out += [c * NT + t for c in range(N_CORES) for t in range(t0, t1)]
    return np.asarray(out, np.int64)


# ---------------------------------------------------------------- host side
def preprocess(x, edge_index, edge_attr,
               W_msg1, Wih1, Whh1, bih1, bhh1,
               W_msg2, Wih2, Whh2, bih2, bhh2, force_k=None):
    """Host-side layout preprocessing. Returns (meta, in_maps)."""
    x = np.asarray(x, np.float32)
    n_nodes, hidden = x.shape
    edge_dim = edge_attr.shape[1]
    assert n_nodes == N_NODES and hidden == P and edge_dim == ED

    src = np.asarray(edge_index[0], np.int64)
    dst = np.asarray(edge_index[1], np.int64)
    ea = np.asarray(edge_attr, np.float32)

    bz = not (np.any(bih1) or np.any(bhh1) or np.any(bih2) or np.any(bhh2))
    assert bz, "nonzero biases not implemented"

    # device node id: each core's shard padded from 2500 to 2560 rows
    src_dev = src + (src // NPC) * (NPC_PAD - NPC)
    core_of = dst // NPC
    dloc = dst - core_of * NPC          # 0..2499 within the owning core

    order = np.lexsort((dloc, core_of))
    core_s, dloc_s, src_s, ea_s = (core_of[order], dloc[order],
                                   src_dev[order], ea[order])
    cstarts = np.searchsorted(core_s, np.arange(N_CORES), side="left")
    cends = np.searchsorted(core_s, np.arange(N_CORES), side="right")

    # y1 = x @ Wx1^T, node-partitioned chunk table in stream order:
    # y1n[p, i*128+f] = y1p[perm[i]*128+p, f]
    perm = chunk_perm()
    y1 = x @ np.asarray(W_msg1)[:, :P].T
    y1p = np.zeros((N_PAD, P), np.float32)
    for c in range(N_CORES):
        y1p[c * NPC_PAD:c * NPC_PAD + NPC] = y1[c * NPC:(c + 1) * NPC]
    y1n = np.ascontiguousarray(
        y1p.reshape(NCH, P, P)[perm].transpose(1, 0, 2)
    ).reshape(P, NCH * P).astype(NPBF16)

    # fp8 byte LUT for small integer counts
    lut = np.asarray(np.arange(256), NPF8).view(np.uint8)

    w_shared = {
        "we1r": np.ascontiguousarray(W_msg1[:, P:].T).astype(NPBF16),
        "wih1t": np.ascontiguousarray(np.asarray(Wih1).T).astype(NPBF16),
        "whh1t": np.ascontiguousarray(np.asarray(Whh1).T).astype(NPBF16),
        "wx2r": np.ascontiguousarray(W_msg2[:, :P].T).astype(NPBF16),
        "we2r": np.ascontiguousarray(W_msg2[:, P:].T).astype(NPBF16),
        "wih2t": np.ascontiguousarray(np.asarray(Wih2).T).astype(NPBF16),
        "whh2t": np.ascontiguousarray(np.asarray(Whh2).T).astype(NPBF16),
        "y1n": y1n,
    }

    in_maps = []
    for c in range(N_CORES):
        sel = slice(cstarts[c], cends[c])
        dl, sr, eac = dloc_s[sel], src_s[sel], ea_s[sel]

        # transposed count matrix M^T[src_dev, dloc] as fp8 bytes, chunks
        # permuted to stream order, grouped [NG, 128, G, 2560]
        mt = np.zeros((N_PAD, NPC_PAD), np.uint8)
        np.add.at(mt, (sr, dl), 1)
        assert mt.max() <= 8, "edge multiplicity too high for exact fp8"
        mtg = np.ascontiguousarray(
            lut[mt].reshape(NCH, P, NPC_PAD)[perm]
            .reshape(NG, G, P, NPC_PAD).transpose(0, 2, 1, 3)
        ).reshape(NG * P, G * NPC_PAD).view(NPF8)

        # A^T = seg_sum(ea, dloc)^T via sorted reduceat  [64, 2560]
        a_c = np.zeros((NPC_PAD, ED), np.float32)
        if len(dl):
            uniq, starts_u = np.unique(dl, return_index=True)
            a_c[uniq] = np.add.reduceat(eac, starts_u, axis=0)
        atT = np.ascontiguousarray(a_c.T).astype(NPBF16)

        rows = np.zeros((NPC_PAD, P), np.float32)
        rows[:NPC] = x[c * NPC:(c + 1) * NPC]
        xsT = np.ascontiguousarray(rows.T)              # [128, 2560]

        im = {"mtg": mtg, "atT": atT, "xsT": xsT.astype(NPBF16)}
        im.update(w_shared)
        in_maps.append(im)
    meta = dict(K=0)
    return meta, in_maps


# ---------------------------------------------------------------- device side
def build(meta, n_iters=1, single_core=False):
    nc = bacc.Bacc("TRN2", target_bir_lowering=False, debug=False,
                   num_devices=1 if single_core else N_CORES)

    mtg_d = nc.dram_tensor("mtg", [NG * P, G * NPC_PAD], F8,
                           kind="ExternalInput")
    y1n_d = nc.dram_tensor("y1n", [P, N_PAD], BF16, kind="ExternalInput")
    atT_d = nc.dram_tensor("atT", [ED, NPC_PAD], BF16, kind="ExternalInput")
    xsT_d = nc.dram_tensor("xsT", [P, NPC_PAD], BF16, kind="ExternalInput")
    w_d = {}
    for nm, shape in [("we1r", [ED, P]), ("wx2r", [P, P]), ("we2r", [ED, P]),
                      ("wih1t", [P, 384]), ("whh1t", [P, 384]),
                      ("wih2t", [P, 384]), ("whh2t", [P, 384])]:
        w_d[nm] = nc.dram_tensor(nm, shape, BF16, kind="ExternalInput")
    # feature-major output; host transposes
    out_d = nc.dram_tensor("out", [P, NPC_PAD], F32, kind="ExternalOutput")

    nwt = [TSPLIT[s + 1] - TSPLIT[s] for s in range(3)]   # tiles per wave

    with tile.TileContext(nc) as tc:
        with tc.tile_pool(name="persist", bufs=1) as pp, \
             tc.tile_pool(name="mtp", bufs=4) as mtp, \
             tc.tile_pool(name="yring", bufs=3) as yp, \
             tc.tile_pool(name="small", bufs=3) as sp, \
             tc.tile_pool(name="psAgg", bufs=1, space="PSUM") as ppsA, \
             tc.tile_pool(name="psG", bufs=2, space="PSUM") as ppsG, \
             tc.tile_pool(name="dram", bufs=1, space="DRAM") as dp:

            # ---- persistent SBUF state
            xsT = pp.tile([P, NPC_PAD], BF16, tag="xsT")
            nc.sync.dma_start(xsT[:], xsT_d[:])
            atT = pp.tile([ED, NPC_PAD], BF16, tag="atT")
            nc.sync.dma_start(atT[:], atT_d[:])
            w = {}
            for nm, h in w_d.items():
                w[nm] = pp.tile(list(h.shape), BF16, tag=nm, name=nm)
                nc.sync.dma_start(w[nm][:], h[:])

            # gathered y2 chunk tables, one per AllGather wave
            ytab = [pp.tile([P, N_CORES * nwt[s] * P], BF16, tag=f"ytab{s}",
                            name=f"ytab{s}") for s in range(3)]
            y2n = pp.tile([P, NPC_PAD], BF16, tag="y2n")   # y2 own chunks
            x1T = pp.tile([P, NPC_PAD], BF16, tag="x1T")   # x1^T own shard
            aggT = pp.tile([P, NPC_PAD], BF16, tag="aggT")
            outT = pp.tile([P, NPC_PAD], F32, tag="outT")

            def stream(wer, ysrc):
                """aggT = We-term + M^T stream. ysrc(g)(j) -> lhsT AP
                [128 src-in-chunk, 128 feat] for chunk slot g*G+j."""
                aggps = [ppsA.tile([P, 512], F32, tag=f"agg{b}",
                                   name=f"agg{b}") for b in range(NB)]
                for b in range(NB):
                    nc.tensor.matmul(aggps[b][:], lhsT=wer[:],
                                     rhs=atT[:, b * 512:(b + 1) * 512],
                                     start=True, stop=False,
                                     skip_group_check=True)
                for g in range(NG):
                    mt = mtp.tile([P, G * NPC_PAD], F8, tag="mt")
                    eng = nc.sync if (g % 2 == 0) else nc.scalar
                    eng.dma_start(mt[:], mtg_d[g * P:(g + 1) * P, :])
                    yt = ysrc(g)
                    for j in range(G):
                        for b in range(NB):
                            nc.tensor.matmul(
                                aggps[b][:],
                                lhsT=yt(j),
                                rhs=mt[:, j * NPC_PAD + b * 512:
                                       j * NPC_PAD + (b + 1) * 512],
                                start=False,
                                stop=(g == NG - 1 and j == G - 1),
                                skip_group_check=True)
                for b in range(NB):
                    nc.vector.tensor_copy(aggT[:, b * 512:(b + 1) * 512],
                                          aggps[b][:])

            def y1_src(g):
                yt = yp.tile([P, G * P], BF16, tag="yt")
                eng = nc.scalar if (g % 2 == 0) else nc.sync
                eng.dma_start(yt[:], y1n_d[:, g * G * P:(g + 1) * G * P])
                return lambda j: yt[:, j * P:(j + 1) * P]

            pm = chunk_perm()

            def y2_src(g):
                def ap(j):
                    ch = int(pm[g * G + j])
                    c, t = divmod(ch, NT)
                    for s in range(3):
                        if t < TSPLIT[s + 1]:
                            col = (c * nwt[s] + (t - TSPLIT[s])) * P
                            return ytab[s][:, col:col + P]
                return ap

            def gru_fm(tag, mT, hT, wihT, whhT, outT_sl):
                """Feature-major GRU for one 128-node dst tile.
                PSUM bank layout: [rT | zT | inT | hnT], each [128, 128].
                outT_sl = n + z * (hT - n), written feature-major."""
                gp = ppsG.tile([P, 512], F32, tag="gru", name=f"g{tag}")
                nc.tensor.matmul(gp[:, 0:P], lhsT=wihT[:, 0:P], rhs=mT,
                                 start=True, stop=False)
                nc.tensor.matmul(gp[:, 0:P], lhsT=whhT[:, 0:P], rhs=hT,
                                 start=False, stop=True)
                nc.tensor.matmul(gp[:, P:256], lhsT=wihT[:, P:256], rhs=mT,
                                 start=True, stop=False, skip_group_check=True)
                nc.tensor.matmul(gp[:, P:256], lhsT=whhT[:, P:256], rhs=hT,
                                 start=False, stop=True, skip_group_check=True)
                nc.tensor.matmul(gp[:, 256:384], lhsT=wihT[:, 256:384],
                                 rhs=mT,
                                 start=True, stop=True, skip_group_check=True)
                nc.tensor.matmul(gp[:, 384:512], lhsT=whhT[:, 256:384],
                                 rhs=hT,
                                 start=True, stop=True, skip_group_check=True)
                rz = sp.tile([P, 256], BF16, tag="rz")
                nc.scalar.activation(rz[:], gp[:, 0:256],
                                     mybir.ActivationFunctionType.Sigmoid)
                tmp = sp.tile([P, P], F32, tag="gtmp")
                nc.vector.tensor_mul(tmp[:], rz[:, 0:P], gp[:, 384:512])
                nc.vector.tensor_add(tmp[:], tmp[:], gp[:, 256:384])
                n_t = sp.tile([P, P], F32, tag="gn")
                nc.scalar.activation(n_t[:], tmp[:],
                                     mybir.ActivationFunctionType.Tanh)
                d_t = sp.tile([P, P], F32, tag="gd")
                nc.vector.tensor_sub(d_t[:], hT, n_t[:])
                nc.vector.tensor_mul(d_t[:], rz[:, P:256], d_t[:])
                nc.vector.tensor_add(outT_sl, n_t[:], d_t[:])

            for it in range(n_iters):
                y2_loc, y2_full = [], []
                for s in range(3):
                    ncol = nwt[s] * P
                    y2_loc.append(dp.tile([P, ncol], BF16,
                                          name=f"y2_loc{s}"))
                    y2_full.append(dp.tile([N_CORES * P, ncol], BF16,
                                           addr_space="Shared",
                                           name=f"y2_full{s}"))

                # ---- layer 1 stream
                stream(w["we1r"], y1_src)

                # ---- GRU1 + y2n + AllGather, in 3 pipelined waves
                for s in range(3):
                    t0, t1 = TSPLIT[s], TSPLIT[s + 1]
                    for t in range(t0, t1):
                        gru_fm(f"1_{it}_{t}", aggT[:, t * P:(t + 1) * P],
                               xsT[:, t * P:(t + 1) * P],
                               w["wih1t"], w["whh1t"],
                               x1T[:, t * P:(t + 1) * P])
                    for h in range(t0 * P // 512, t1 * P // 512):
                        psy = ppsG.tile([P, 512], F32, tag="gru",
                                        name=f"psy{it}{h}")
                        for q in range(4):
                            t = h * 4 + q
                            nc.tensor.matmul(
                                psy[:, q * P:(q + 1) * P],
                                lhsT=x1T[:, t * P:(t + 1) * P],
                                rhs=w["wx2r"][:],
                                start=True, stop=True,
                                skip_group_check=True)
                        nc.vector.tensor_copy(
                            y2n[:, h * 512:(h + 1) * 512], psy[:])
                    nc.sync.dma_start(y2_loc[s][:],
                                      y2n[:, TSPLIT[s] * P:TSPLIT[s + 1] * P])
                    if single_core:
                        nc.sync.dma_start(y2_full[s][0:P, :], y2_loc[s][:])
                    else:
                        nc.gpsimd.collective_compute(
                            "AllGather", mybir.AluOpType.bypass,
                            replica_groups=[list(range(N_CORES))],
                            ins=[y2_loc[s][:].opt()],
                            outs=[y2_full[s][:].opt()])

                # ---- spread the gathered tables into SBUF
                nblk = 1 if single_core else N_CORES
                for s in range(3):
                    ncol = nwt[s] * P
                    for c in range(nblk):
                        eng = nc.sync if (c % 2 == 0) else nc.scalar
                        eng.dma_start(ytab[s][:, c * ncol:(c + 1) * ncol],
                                      y2_full[s][c * P:(c + 1) * P, :])

                # ---- layer 2 stream + GRU2 -> feature-major out
                stream(w["we2r"], y2_src)
                for t in range(NT):
                    gru_fm(f"2_{it}_{t}", aggT[:, t * P:(t + 1) * P],
                           x1T[:, t * P:(t + 1) * P],
                           w["wih2t"], w["whh2t"],
                           outT[:, t * P:(t + 1) * P])
                nc.sync.dma_start(out_d[:], outT[:])

    nc.compile()
    return nc


# ---------------------------------------------------------------- entry point
_CACHE = {}


def kernel(**inputs) -> np.ndarray:
    """Full (unsharded) inputs in, full [N, 128] float32 output out."""
    from concourse import bass_utils

    meta, in_maps = preprocess(**inputs)
    key = ("v6",)
    nc = _CACHE.get(key)
    if nc is None:
        nc = build(meta)
        _CACHE[key] = nc
    res = bass_utils.run_bass_kernel_spmd(nc, in_maps,
                                          core_ids=list(range(N_CORES)))
    out = np.stack([res.results[c]["out"][:, :NPC].T
                    for c in range(N_CORES)], axis=0).reshape(N_NODES, P)
    return np.ascontiguousarray(out, dtype=np.float32)
